# revision 11
# baseline (speedup 1.0000x reference)
"""EGNN layer on 8 Trainium2 NeuronCores.

Sort edges by destination (row); partition nodes into 8 contiguous ranges of
6250 (one per core), each core takes the edges landing in its range. Edges are
grouped per 128-node block so segment-sums accumulate in PSUM via one-hot
matmuls -- no collectives. Each core holds the full [h|pos] table for source
gathers (indirect DMA). One SPMD program; per-core data differs.
"""
import os
import numpy as np
from contextlib import ExitStack

N = 50000
E = 800000
D = 128
ED = 51
NC = 8
NCORE = N // NC               # 6250
NBLK = (NCORE + 127) // 128   # 49
LASTB = NCORE - 128 * (NBLK - 1)  # 106
DW = 132                      # 128 h + 3 pos + 1 pad
G = 4                         # tiles (of 128 edges) per gather group


def _host_prep(h, edge_index, edge_attr, pos):
    row = np.asarray(edge_index[0], dtype=np.int64)
    col = np.asarray(edge_index[1], dtype=np.int64)
    order = np.argsort(row, kind="stable")
    row_s = row[order].astype(np.int32)
    col_s = col[order].astype(np.int32)
    ea_s = np.asarray(edge_attr, dtype=np.float32)[order]

    core = row_s // NCORE
    local = row_s % NCORE
    blk = local // 128
    j = local % 128
    gb = core * NBLK + blk
    counts = np.bincount(gb, minlength=NC * NBLK)
    T_blk = int((counts.max() + 127) // 128)
    E_blk = T_blk * 128
    E_prog = NBLK * E_blk

    starts = np.zeros(NC * NBLK, dtype=np.int64)
    np.cumsum(counts[:-1], out=starts[1:])
    within = np.arange(len(row_s), dtype=np.int64) - starts[gb]
    slot = blk.astype(np.int64) * E_blk + within

    gidx_r = np.zeros((NC, E_prog), dtype=np.int32)
    gidx_c = np.zeros((NC, E_prog), dtype=np.int32)
    rloc = np.full((NC, E_prog), -1.0, dtype=np.float32)
    eaT = np.zeros((NC, ED, E_prog), dtype=np.float32)
    for c in range(NC):
        m = core == c
        s = slot[m]
        gidx_r[c, s] = row_s[m]
        gidx_c[c, s] = col_s[m]
        rloc[c, s] = j[m].astype(np.float32)
        eaT[c][:, s] = ea_s[m].T

    hpos = np.zeros((N, DW), dtype=np.float32)
    hpos[:, :128] = h
    hpos[:, 128:131] = pos

    def tilize(a, dt):
        return np.ascontiguousarray(
            a.reshape(NC, NBLK * T_blk, 128).transpose(0, 2, 1)).astype(dt)

    idx_r = tilize(gidx_r, np.int32)
    idx_c = tilize(gidx_c, np.int32)
    rloc_t = tilize(rloc, np.float32)

    hTo = np.ascontiguousarray(
        np.asarray(h, np.float32).reshape(NC, NCORE, 128).transpose(0, 2, 1))
    pp = np.zeros((NC, NBLK * 128, 3), dtype=np.float32)
    pp[:, :NCORE] = np.asarray(pos, np.float32).reshape(NC, NCORE, 3)
    poso = np.ascontiguousarray(
        pp.reshape(NC, NBLK, 128, 3).transpose(0, 2, 1, 3).reshape(NC, 128, NBLK * 3))
    return dict(T_blk=T_blk, E_prog=E_prog, hpos=hpos, idx_r=idx_r, idx_c=idx_c,
                rloc=rloc_t, eaT=np.ascontiguousarray(eaT), hTo=hTo, poso=poso)


def _build_program(T_blk):
    import concourse.bass as bass
    import concourse.mybir as mybir
    import concourse.tile as tile
    from concourse import bacc

    f32 = mybir.dt.float32
    i32 = mybir.dt.int32
    ALU = mybir.AluOpType
    ACTF = mybir.ActivationFunctionType
    NT = NBLK * T_blk
    E_prog = NT * 128

    nc = bacc.Bacc()
    hpos = nc.dram_tensor("hpos", [N, DW], f32, kind="ExternalInput")
    idxr = nc.dram_tensor("idxr", [128, NT], i32, kind="ExternalInput")
    idxc = nc.dram_tensor("idxc", [128, NT], i32, kind="ExternalInput")
    rloc = nc.dram_tensor("rloc", [128, NT], f32, kind="ExternalInput")
    eaT = nc.dram_tensor("eaT", [ED, E_prog], f32, kind="ExternalInput")
    hTo = nc.dram_tensor("hTo", [128, NCORE], f32, kind="ExternalInput")
    poso = nc.dram_tensor("poso", [128, NBLK * 3], f32, kind="ExternalInput")
    ident = nc.dram_tensor("ident", [128, 128], f32, kind="ExternalInput")
    iota = nc.dram_tensor("iota", [128, 128], f32, kind="ExternalInput")
    Wm = {}
    for nm, shp in [("W1a", [128, 128]), ("W1b", [128, 128]), ("W1c", [ED, 128]),
                    ("Wr", [1, 128]), ("W2", [128, 128]), ("Wc1", [128, 128]),
                    ("Wc2", [128, 1]), ("Wn1a", [128, 128]), ("Wn1b", [128, 128]),
                    ("Wn2", [128, 128]), ("be1", [128, 1]), ("be2", [128, 1]),
                    ("bc1", [128, 1]), ("bn1", [128, 1]), ("bn2", [128, 1])]:
        Wm[nm] = nc.dram_tensor(nm, shp, f32, kind="ExternalInput")
    houtT = nc.dram_tensor("houtT", [128, NCORE], f32, kind="ExternalOutput")
    posout = nc.dram_tensor("posout", [128, NBLK * 3], f32, kind="ExternalOutput")

    # group sizes per block: T_blk tiles split into chunks of <= G
    gsizes = [G] * (T_blk // G) + ([T_blk % G] if T_blk % G else [])

    with tile.TileContext(nc) as tc, ExitStack() as ctx:
        cst = ctx.enter_context(tc.tile_pool(name="cst", bufs=1))
        gth = ctx.enter_context(tc.tile_pool(name="gth", bufs=3))
        eap = ctx.enter_context(tc.tile_pool(name="eap", bufs=3))
        wrk = ctx.enter_context(tc.tile_pool(name="wrk", bufs=2))
        sml = ctx.enter_context(tc.tile_pool(name="sml", bufs=3))
        outp = ctx.enter_context(tc.tile_pool(name="outp", bufs=2))
        psT = ctx.enter_context(tc.tile_pool(name="psT", bufs=3, space="PSUM"))
        psM = ctx.enter_context(tc.tile_pool(name="psM", bufs=2, space="PSUM"))
        psG = ctx.enter_context(tc.tile_pool(name="psG", bufs=1, space="PSUM"))
        psP = ctx.enter_context(tc.tile_pool(name="psP", bufs=1, space="PSUM"))

        W = {k: cst.tile_from(v[:, :], name=k) for k, v in Wm.items()}
        id_sb = cst.tile_from(ident[:, :])
        io_sb = cst.tile_from(iota[:, :])
        idxr_sb = cst.tile_from(idxr[:, :])
        idxc_sb = cst.tile_from(idxc[:, :])
        rloc_sb = cst.tile_from(rloc[:, :])
        hTo_sb = cst.tile_from(hTo[:, :])
        poso_sb = cst.tile_from(poso[:, :])

        for b in range(NBLK):
            blkn = 128 if b < NBLK - 1 else LASTB
            aggH = psG.tile([128, 128], f32, tag="aggH")
            aggP = psP.tile([128, 3], f32, tag="aggP")
            t0 = b * T_blk
            first = True
            for gi, g in enumerate(gsizes):
                c0 = t0 + sum(gsizes[:gi])       # first tile col of group
                e0 = c0 * 128                     # first edge slot
                # --- gathers ---
                grow = gth.tile([128, g * DW], f32, tag="grow")
                gcol = gth.tile([128, g * DW], f32, tag="gcol")
                for t in range(g):
                    nc.gpsimd.indirect_dma_start(
                        out=grow[:, t * DW:(t + 1) * DW], out_offset=None, in_=hpos[:, :],
                        in_offset=bass.IndirectOffsetOnAxis(ap=idxr_sb[:, c0 + t:c0 + t + 1], axis=0))
                    nc.gpsimd.indirect_dma_start(
                        out=gcol[:, t * DW:(t + 1) * DW], out_offset=None, in_=hpos[:, :],
                        in_offset=bass.IndirectOffsetOnAxis(ap=idxc_sb[:, c0 + t:c0 + t + 1], axis=0))
                ea_sb = eap.tile([ED, g * 128], f32, tag="ea")
                nc.sync.dma_start(ea_sb[:, :], eaT[:, e0:e0 + g * 128])

                g3r = grow[:].rearrange("p (g d) -> p g d", g=g)
                g3c = gcol[:].rearrange("p (g d) -> p g d", g=g)

                # --- geometry (edge-major) ---
                diff = sml.tile([128, g, 3], f32, tag="diff")
                nc.vector.tensor_tensor(out=diff[:], in0=g3r[:, :, 128:131],
                                        in1=g3c[:, :, 128:131], op=ALU.subtract)
                dsq = sml.tile([128, g, 3], f32, tag="dsq")
                nc.vector.tensor_tensor(out=dsq[:], in0=diff[:], in1=diff[:], op=ALU.mult)
                rsq = sml.tile([128, g], f32, tag="rsq")
                nc.vector.tensor_reduce(out=rsq[:].rearrange("p (g o) -> p g o", o=1),
                                        in_=dsq[:], axis=mybir.AxisListType.X, op=ALU.add)
                rad = sml.tile([128, g], f32, tag="rad")
                nc.scalar.activation(out=rad[:], in_=rsq[:], func=ACTF.Sqrt)
                radT = sml.tile([1, g * 128], f32, tag="radT")
                for t in range(g):
                    radT_ps = psT.tile([1, 128], f32, tag="tpr", bufs=1)
                    nc.tensor.transpose(out=radT_ps[:], in_=rad[:, t:t + 1],
                                        identity=id_sb[:])
                    nc.vector.tensor_copy(out=radT[:, t * 128:(t + 1) * 128],
                                          in_=radT_ps[:])

                # --- transpose h_row / h_col to [feat, edge] ---
                hrT_ps = psT.tile([128, g * 128], f32, tag="tp", bufs=2)
                for t in range(g):
                    nc.tensor.transpose(out=hrT_ps[:, t * 128:(t + 1) * 128],
                                        in_=g3r[:, t, 0:128], identity=id_sb[:])
                hrT = wrk.tile([128, g * 128], f32, tag="hrT")
                nc.vector.tensor_copy(out=hrT[:], in_=hrT_ps[:])
                hcT_ps = psT.tile([128, g * 128], f32, tag="tp", bufs=2)
                for t in range(g):
                    nc.tensor.transpose(out=hcT_ps[:, t * 128:(t + 1) * 128],
                                        in_=g3c[:, t, 0:128], identity=id_sb[:])
                hcT = wrk.tile([128, g * 128], f32, tag="hcT")
                nc.vector.tensor_copy(out=hcT[:], in_=hcT_ps[:])

                # --- edge MLP (feat-major) ---
                e1p = psM.tile([128, g * 128], f32, tag="mm")
                nc.tensor.matmul(out=e1p[:], lhsT=W["W1a"][:], rhs=hrT[:], start=True, stop=False)
                nc.tensor.matmul(out=e1p[:], lhsT=W["W1b"][:], rhs=hcT[:], start=False, stop=False)
                nc.tensor.matmul(out=e1p[:], lhsT=W["W1c"][:], rhs=ea_sb[:], start=False, stop=False)
                for t in range(g):
                    nc.tensor.matmul(out=e1p[:, t * 128:(t + 1) * 128], lhsT=W["Wr"][:],
                                     rhs=radT[0:1, t * 128:(t + 1) * 128],
                                     start=False, stop=(t == g - 1))
                e1 = wrk.tile([128, g * 128], f32, tag="e1")
                nc.scalar.activation(out=e1[:], in_=e1p[:], func=ACTF.Silu, bias=W["be1"][:, :])
                e2p = psM.tile([128, g * 128], f32, tag="mm")
                nc.tensor.matmul(out=e2p[:], lhsT=W["W2"][:], rhs=e1[:], start=True, stop=True)
                e2 = wrk.tile([128, g * 128], f32, tag="e2")
                nc.scalar.activation(out=e2[:], in_=e2p[:], func=ACTF.Silu, bias=W["be2"][:, :])
                c1p = psM.tile([128, g * 128], f32, tag="mm")
                nc.tensor.matmul(out=c1p[:], lhsT=W["Wc1"][:], rhs=e2[:], start=True, stop=True)
                c1 = wrk.tile([128, g * 128], f32, tag="c1")
                nc.scalar.activation(out=c1[:], in_=c1p[:], func=ACTF.Silu, bias=W["bc1"][:, :])
                cup = psM.tile([1, g * 128], f32, tag="cu", bufs=1)
                nc.tensor.matmul(out=cup[:], lhsT=W["Wc2"][:], rhs=c1[:], start=True, stop=True)
                cu = sml.tile([1, g * 128], f32, tag="cuc")
                nc.vector.tensor_scalar(out=cu[:], in0=cup[:], scalar1=1.0,
                                        scalar2=-1.0, op0=ALU.min, op1=ALU.max)

                # --- back to edge-major: e2, cu ---
                e2e_ps = psT.tile([128, g * 128], f32, tag="tp", bufs=2)
                for t in range(g):
                    nc.tensor.transpose(out=e2e_ps[:, t * 128:(t + 1) * 128],
                                        in_=e2[:, t * 128:(t + 1) * 128], identity=id_sb[:])
                e2e = wrk.tile([128, g * 128], f32, tag="e2e")
                nc.vector.tensor_copy(out=e2e[:], in_=e2e_ps[:])
                cue_ps = psT.tile([128, g], f32, tag="tp", bufs=2)
                for t in range(g):
                    nc.tensor.matmul(out=cue_ps[:, t:t + 1],
                                     lhsT=cu[:, t * 128:(t + 1) * 128],
                                     rhs=id_sb[0:1, 0:1], start=True, stop=True)
                cue = sml.tile([128, g], f32, tag="cue")
                nc.vector.tensor_copy(out=cue[:], in_=cue_ps[:])
                trans = sml.tile([128, g, 3], f32, tag="trans")
                for t in range(g):
                    nc.vector.tensor_tensor(out=trans[:, t, :], in0=diff[:, t, :],
                                            in1=cue[:, t:t + 1].to_broadcast([128, 3]),
                                            op=ALU.mult)

                # --- one-hot S and segment matmuls ---
                S = wrk.tile([128, g * 128], f32, tag="S")
                for t in range(g):
                    nc.vector.tensor_tensor(out=S[:, t * 128:(t + 1) * 128], in0=io_sb[:],
                                            in1=rloc_sb[:, c0 + t:c0 + t + 1].to_broadcast([128, 128]),
                                            op=ALU.is_equal)
                last_g = gi == len(gsizes) - 1
                for t in range(g):
                    lt = last_g and t == g - 1
                    nc.tensor.matmul(out=aggH[:], lhsT=S[:, t * 128:(t + 1) * 128],
                                     rhs=e2e[:, t * 128:(t + 1) * 128],
                                     start=first, stop=lt)
                    nc.tensor.matmul(out=aggP[:], lhsT=S[:, t * 128:(t + 1) * 128],
                                     rhs=trans[:, t, :], start=first, stop=lt)
                    first = False

            # --- node MLP for this block ---
            aggHs = wrk.tile([128, 128], f32, tag="aggHs")
            nc.vector.tensor_copy(out=aggHs[:], in_=aggH[:])
            aggT_ps = psT.tile([128, 128], f32, tag="tp", bufs=2)
            nc.tensor.transpose(out=aggT_ps[:], in_=aggHs[:], identity=id_sb[:])
            aggT = wrk.tile([128, 128], f32, tag="aggT")
            nc.vector.tensor_copy(out=aggT[:], in_=aggT_ps[:])
            n1p = psM.tile([128, 128], f32, tag="mm")
            nc.tensor.matmul(out=n1p[:, :blkn], lhsT=W["Wn1a"][:],
                             rhs=hTo_sb[:, b * 128:b * 128 + blkn], start=True, stop=False)
            nc.tensor.matmul(out=n1p[:, :blkn], lhsT=W["Wn1b"][:],
                             rhs=aggT[:, :blkn], start=False, stop=True)
            n1 = wrk.tile([128, 128], f32, tag="n1")
            nc.scalar.activation(out=n1[:, :blkn], in_=n1p[:, :blkn], func=ACTF.Silu,
                                 bias=W["bn1"][:, :])
            o2p = psM.tile([128, 128], f32, tag="mm")
            nc.tensor.matmul(out=o2p[:, :blkn], lhsT=W["Wn2"][:], rhs=n1[:, :blkn],
                             start=True, stop=True)
            ho = outp.tile([128, 128], f32, tag="ho")
            nc.vector.tensor_tensor(out=ho[:, :blkn], in0=o2p[:, :blkn],
                                    in1=W["bn2"][:, 0:1].to_broadcast([128, blkn]),
                                    op=ALU.add)
            nc.vector.tensor_tensor(out=ho[:, :blkn], in0=ho[:, :blkn],
                                    in1=hTo_sb[:, b * 128:b * 128 + blkn], op=ALU.add)
            nc.sync.dma_start(houtT[:, b * 128:b * 128 + blkn], ho[:, :blkn])
            po = outp.tile([128, 3], f32, tag="po")
            nc.vector.tensor_tensor(out=po[:], in0=aggP[:],
                                    in1=poso_sb[:, b * 3:(b + 1) * 3], op=ALU.add)
            nc.sync.dma_start(posout[:, b * 3:(b + 1) * 3], po[:, :])
    nc.compile()
    return nc


def kernel(h, edge_index, edge_attr, pos,
           W_e1, b_e1, W_e2, b_e2, W_c1, b_c1, W_c2, W_n1, b_n1, W_n2, b_n2):
    from concourse.bass_utils import run_bass_kernel_spmd

    h = np.asarray(h, np.float32)
    pos = np.asarray(pos, np.float32)
    prep = _host_prep(h, edge_index, edge_attr, pos)
    T_blk = prep["T_blk"]
    nc = _build_program(T_blk)

    W_e1 = np.asarray(W_e1, np.float32)
    W_n1 = np.asarray(W_n1, np.float32)
    shared = dict(
        hpos=prep["hpos"],
        ident=np.eye(128, dtype=np.float32),
        iota=np.ascontiguousarray(
            np.broadcast_to(np.arange(128, dtype=np.float32), (128, 128))),
        W1a=np.ascontiguousarray(W_e1[0:128]),
        W1b=np.ascontiguousarray(W_e1[128:256]),
        W1c=np.ascontiguousarray(W_e1[256:307]),
        Wr=np.ascontiguousarray(W_e1[307:308]),
        W2=np.asarray(W_e2, np.float32),
        Wc1=np.asarray(W_c1, np.float32),
        Wc2=np.asarray(W_c2, np.float32),
        Wn1a=np.ascontiguousarray(W_n1[0:128]),
        Wn1b=np.ascontiguousarray(W_n1[128:256]),
        Wn2=np.asarray(W_n2, np.float32),
        be1=np.asarray(b_e1, np.float32).reshape(128, 1),
        be2=np.asarray(b_e2, np.float32).reshape(128, 1),
        bc1=np.asarray(b_c1, np.float32).reshape(128, 1),
        bn1=np.asarray(b_n1, np.float32).reshape(128, 1),
        bn2=np.asarray(b_n2, np.float32).reshape(128, 1),
    )
    in_maps = []
    for c in range(NC):
        m = dict(shared)
        m.update(idxr=prep["idx_r"][c], idxc=prep["idx_c"][c], rloc=prep["rloc"][c],
                 eaT=prep["eaT"][c], hTo=prep["hTo"][c], poso=prep["poso"][c])
        in_maps.append(m)

    import time as _time
    t0 = _time.perf_counter()
    res = run_bass_kernel_spmd(nc, in_maps, core_ids=list(range(NC)))
    kernel.last_run_wall_s = _time.perf_counter() - t0
    kernel.last_exec_time_ns = getattr(res, "exec_time_ns", None)
    h_new = np.empty((N, D), np.float32)
    pos_new = np.empty((N, 3), np.float32)
    for c in range(NC):
        h_new[c * NCORE:(c + 1) * NCORE] = res.results[c]["houtT"].T
        pp = res.results[c]["posout"].reshape(128, NBLK, 3).transpose(1, 0, 2)
        pos_new[c * NCORE:(c + 1) * NCORE] = pp.reshape(-1, 3)[:NCORE]
    return (h_new, pos_new)
